# revision 1
# baseline (speedup 1.0000x reference)
"""Dice-score kernel for TRN2 (8 NeuronCores, SPMD row-sharded).

Math (matches reference):
    pred = argmax(output, axis=1)            # (V,) in {0..3}
    o    = pred[segments]                    # per-pixel gather
    inter[c] = 2*|{t==c & o==c}| ; union[c] = |{t==c}| + |{o==c}|
    score = inter / (union + 1e-10)

Device strategy per core (512 rows = 2,097,152 pixels, viewed (128, 16384)):
  - GPSIMD ap_gather with a 16384-entry int32 pred table (replicated per
    partition) produces o in "wrapped stream" layout (16x replicated per
    16-partition group).
  - The stream diagonal (partition p = 16g+r, free 16s+r) is exactly the
    natural layout, so 16 strided copies (10 on ACT, 6 on DVE) extract
    o_nat aligned with t.
  - DVE computes 10 running sums via accum_out:
      St1=sum t, St2=sum t^2, Stm=sum min(t,1),
      Su =sum u (u = [t==o]), So1, So2, Som,
      Su1=sum u*o, Su2=sum u*o^2, Sum=sum u*min(o,1)
  - Host inverts the tiny 4x4 systems [1, c, c^2, min(c,1)] to get the
    4-bin counts, then forms the dice score.
"""

import os
import sys

sys.path.insert(0, "/opt/trn_rl_repo")
# The GPSIMD gather's strided diagonal readers defeat subtile overlap
# analysis (missed RAW edge); track dependencies at whole-tile granularity.
os.environ["BY_DEFAULT_DISABLE_SUBTILE_DEPS"] = "1"

from contextlib import ExitStack

import numpy as np

import concourse.bass as bass
import concourse.tile as tile
from concourse import bacc, mybir

NCORES = 8
V = 16384
NCLS = 4
N = 4096
ROWS = N // NCORES            # 512 rows per core
PIX = ROWS * N                # 2097152 pixels per core
PPART = PIX // 128            # 16384 pixels per partition
FT = 512                      # natural free slots per tile
NT = PPART // FT              # 32 tiles
NIDX = 16 * FT                # 8192 stream indices per gather
NMOM = 10
NACT_DIAG = 10                # diagonal residues handled by ScalarE (rest on DVE)

i32 = mybir.dt.int32
i16 = mybir.dt.int16
f32 = mybir.dt.float32
bf16 = mybir.dt.bfloat16


def _build_program():
    nc = bacc.Bacc(
        "TRN2", target_bir_lowering=False, debug=False, num_devices=NCORES
    )
    outp = nc.dram_tensor("outp", [128, 128, NCLS], f32, kind="ExternalInput")
    targ = nc.dram_tensor("targ", [128, PPART], i32, kind="ExternalInput")
    segs = nc.dram_tensor("segs", [128, PPART, 2], i16, kind="ExternalInput")
    wde = nc.dram_tensor("wde", [128, 16 * 128], bf16, kind="ExternalInput")
    mom = nc.dram_tensor("mom", [128, NMOM], f32, kind="ExternalOutput")

    with tile.TileContext(nc) as tc:
        with ExitStack() as ctx:
            _kernel(ctx, tc, nc, outp, targ, segs, wde, mom)

    nc.compile()
    return nc


def _kernel(ctx, tc, nc, outp, targ, segs, wde, mom):
    from concourse.alu_op_type import AluOpType as Op

    const_pool = ctx.enter_context(tc.tile_pool(name="const", bufs=1))
    dram_pool = ctx.enter_context(tc.tile_pool(name="dram", bufs=1, space="DRAM"))
    pred_pool = ctx.enter_context(tc.tile_pool(name="predp", bufs=2))
    in_pool = ctx.enter_context(tc.tile_pool(name="inp", bufs=3))
    stream_pool = ctx.enter_context(tc.tile_pool(name="stream", bufs=2))
    nat_pool = ctx.enter_context(tc.tile_pool(name="nat", bufs=2))
    tmp_pool = ctx.enter_context(tc.tile_pool(name="tmp", bufs=2))
    psum_pool = ctx.enter_context(tc.tile_pool(name="ps", bufs=2, space="PSUM"))

    # ---- Phase 0: pred = argmax(output, axis=1), built into a gather table --
    o_all = pred_pool.tile([128, 128, NCLS], f32)
    nc.sync.dma_start(o_all, outp.ap())

    best = pred_pool.tile([128, 128, 1], f32, tag="best")
    pred = pred_pool.tile([128, 128, 1], i32, tag="pred")
    nc.vector.tensor_copy(best, o_all[:, :, 0:1])
    nc.vector.memset(pred, 0)
    for c in range(1, NCLS):
        oc = o_all[:, :, c : c + 1]
        gt = pred_pool.tile([128, 128, 1], i32, tag="gt")
        nc.vector.tensor_tensor(gt, oc, best, Op.is_gt)
        cst = pred_pool.tile([128, 128, 1], i32, tag="cst")
        nc.vector.memset(cst, c)
        nc.vector.copy_predicated(pred, gt, cst)
        best2 = pred_pool.tile([128, 128, 1], f32, tag="best")
        nc.vector.tensor_tensor(best2, best, oc, Op.max)
        best = best2

    # table values as fp32 so the de-group matmul output is exact
    predf = pred_pool.tile([128, 128, 1], f32, tag="predf")
    nc.vector.tensor_copy(predf, pred)
    pred_scr = dram_pool.tile([128, 128], f32)
    nc.sync.dma_start(pred_scr, predf)

    # Broadcast the 16384-entry table into every partition (stride-0 source).
    tbl = const_pool.tile([128, V], f32)
    scr_flat = bass.AP(pred_scr.tensor, pred_scr.offset, [[0, 128], [1, V]])
    nc.sync.dma_start(tbl, scr_flat)

    # De-group weights (host-built constant), one 128x128 block per stream
    # residue q: W_q[p, j] = 1/16 where j in [8q, 8q+8) and p//16 == j - 8q.
    wtile = const_pool.tile([128, 16 * 128], bf16)
    nc.sync.dma_start(wtile, wde.ap())
    wdes = [wtile[:, 128 * q : 128 * (q + 1)] for q in range(16)]

    # ---- Accumulator strip: one fp32 column per (moment, tile) -------------
    acc = const_pool.tile([128, NMOM * NT], f32)

    # ---- Phase 1: main loop ------------------------------------------------
    for it in range(NT):
        seg16 = in_pool.tile([128, FT], i16, tag="seg")
        nc.sync.dma_start(seg16, segs.ap()[:, it * FT : (it + 1) * FT, 0:1])
        # t in "q-major" layout: partition p = 8q+m holds HBM chunk 16m+q
        t2 = in_pool.tile([128, FT], i32, tag="t")
        tsrc = bass.AP(
            targ.ap().tensor,
            it * FT,
            [[PPART, 16], [16 * PPART, 8], [1, FT]],
        )
        nc.sync.dma_start(t2, tsrc)

        ostr = stream_pool.tile([128, NIDX], i32, tag="ostr")
        ostr_f = ostr.bitcast(f32)
        nc.gpsimd.ap_gather(
            ostr_f, tbl, seg16, channels=128, num_elems=V, d=1, num_idxs=NIDX
        )

        # De-group: for each stream residue q, one matmul extracts each
        # pixel's o exactly once into psum (8, FT), then DMA reshapes it
        # into partitions [8q, 8q+16) of the natural o_nat tile.
        o_nat = nat_pool.tile([128, FT], f32, tag="onat")
        # bf16 view of the fp32 stream: the high half of each fp32 word is
        # exactly bf16 for the small-int table values.
        ostr_bf = ostr.bitcast(bf16).rearrange("p (s x) -> p s x", x=32)
        psq = psum_pool.tile([128, FT], f32, tag="psq")
        for q in range(16):
            nc.tensor.matmul(
                psq,
                wdes[q],
                ostr_bf[:, :, 2 * q + 1 : 2 * q + 2],
                start=(q == 0),
                stop=(q == 15),
            )
        nc.scalar.copy(o_nat, psq)

        def a(m):
            k = m * NT + it
            return acc[:, k : k + 1]

        # ---- t moments ----
        t2f = tmp_pool.tile([128, FT], f32, tag="t2f")
        nc.vector.tensor_copy(t2f, t2)
        w0 = tmp_pool.tile([128, FT], f32, tag="w", bufs=4)
        nc.vector.tensor_scalar(w0, t2f, 0.0, None, Op.add, Op.add, accum_out=a(0))
        w1 = tmp_pool.tile([128, FT], f32, tag="w", bufs=4)
        nc.vector.scalar_tensor_tensor(
            w1, t2f, 0.0, t2f, Op.bypass, Op.mult, accum_out=a(1)
        )
        w2 = tmp_pool.tile([128, FT], f32, tag="w", bufs=4)
        nc.vector.tensor_scalar(w2, t2f, 1.0, None, Op.min, Op.add, accum_out=a(2))

        # ---- u = (t == o) ----
        u = tmp_pool.tile([128, FT], f32, tag="u")
        nc.vector.scalar_tensor_tensor(
            u, t2f, 0.0, o_nat, Op.bypass, Op.is_equal, accum_out=a(3)
        )

        # ---- o moments ----
        w3 = tmp_pool.tile([128, FT], f32, tag="w", bufs=4)
        nc.vector.tensor_scalar(w3, o_nat, 0.0, None, Op.add, Op.add, accum_out=a(4))
        w4 = tmp_pool.tile([128, FT], f32, tag="w", bufs=4)
        nc.vector.scalar_tensor_tensor(
            w4, o_nat, 0.0, o_nat, Op.bypass, Op.mult, accum_out=a(5)
        )
        mo = tmp_pool.tile([128, FT], f32, tag="mo")
        nc.vector.tensor_scalar(mo, o_nat, 1.0, None, Op.min, Op.add, accum_out=a(6))

        # ---- u-restricted o moments ----
        uo = tmp_pool.tile([128, FT], f32, tag="uo")
        nc.vector.scalar_tensor_tensor(
            uo, u, 0.0, o_nat, Op.bypass, Op.mult, accum_out=a(7)
        )
        w5 = tmp_pool.tile([128, FT], f32, tag="w", bufs=4)
        nc.vector.scalar_tensor_tensor(
            w5, uo, 0.0, o_nat, Op.bypass, Op.mult, accum_out=a(8)
        )
        w6 = tmp_pool.tile([128, FT], f32, tag="w", bufs=4)
        nc.vector.scalar_tensor_tensor(
            w6, u, 0.0, mo, Op.bypass, Op.mult, accum_out=a(9)
        )

    # ---- Phase 2: fold the per-tile partials and ship out ------------------
    mom_sb = const_pool.tile([128, NMOM], f32)
    for m in range(NMOM):
        nc.vector.tensor_reduce(
            mom_sb[:, m : m + 1],
            acc[:, m * NT : (m + 1) * NT],
            mybir.AxisListType.X,
            Op.add,
        )
    nc.sync.dma_start(mom.ap(), mom_sb)


_program = None


def _get_program():
    global _program
    if _program is None:
        _program = _build_program()
    return _program


def _make_in_maps(output, target, segments):
    in_maps = []
    for c in range(NCORES):
        tblk = np.ascontiguousarray(target[c * ROWS : (c + 1) * ROWS]).reshape(
            128, PPART
        )
        sblk = np.ascontiguousarray(segments[c * ROWS : (c + 1) * ROWS]).reshape(
            128, PPART
        )
        s16 = sblk.view(np.int16).reshape(128, PPART, 2)
        in_maps.append(
            {
                "outp": np.ascontiguousarray(output).reshape(128, 128, NCLS),
                "targ": tblk,
                "segs": s16,
                "wde": _wde_const(),
            }
        )
    return in_maps


_wde_cache = None


def _wde_const():
    global _wde_cache
    if _wde_cache is None:
        import ml_dtypes

        w = np.zeros((128, 16, 128), dtype=np.float32)
        for q in range(16):
            for m in range(8):
                w[16 * m : 16 * (m + 1), q, 8 * q + m] = 1.0 / 16.0
        _wde_cache = w.reshape(128, 16 * 128).astype(ml_dtypes.bfloat16)
    return _wde_cache


# Basis matrix: rows are sums of [1, c, c^2, min(c,1)] over classes c=0..3.
_M = np.array(
    [
        [1.0, 1.0, 1.0, 1.0],
        [0.0, 1.0, 2.0, 3.0],
        [0.0, 1.0, 4.0, 9.0],
        [0.0, 1.0, 1.0, 1.0],
    ]
)


def _score_from_moments(s, p_total):
    # s: (10,) float64 summed over cores and partitions
    st = np.array([p_total, s[0], s[1], s[2]])
    so = np.array([p_total, s[4], s[5], s[6]])
    su = np.array([s[3], s[7], s[8], s[9]])
    nt = np.linalg.solve(_M, st)
    no = np.linalg.solve(_M, so)
    ju = np.linalg.solve(_M, su)
    score = 2.0 * ju / (nt + no + 1e-10)
    return score.astype(np.float32)


def kernel(output, target, segments):
    from concourse.bass_utils import run_bass_kernel_spmd

    nc = _get_program()
    in_maps = _make_in_maps(output, target, segments)
    res = run_bass_kernel_spmd(nc, in_maps, core_ids=list(range(NCORES)))
    s = np.zeros(NMOM, dtype=np.float64)
    for core_out in res.results:
        s += core_out["mom"].astype(np.float64).sum(axis=0)
    return _score_from_moments(s, float(NCORES * PIX))



# revision 2
# speedup vs baseline: 1.0066x; 1.0066x over previous
"""Dice-score kernel for TRN2 (8 NeuronCores, SPMD row-sharded).

Math (matches reference):
    pred = argmax(output, axis=1)            # (V,) in {0..3}
    o    = pred[segments]                    # per-pixel gather
    inter[c] = 2*|{t==c & o==c}| ; union[c] = |{t==c}| + |{o==c}|
    score = inter / (union + 1e-10)

Device strategy per core (512 rows = 2,097,152 pixels, viewed (128, 16384)):
  - Contiguous int32 DMA loads for target/segments (the v1 kernel used
    2-byte-stride-4 element DMAs which exploded into 65536 descriptors
    per tile and made the whole kernel DMA-descriptor-bound).
  - GPSIMD narrows segments to int16 and runs ap_gather against a
    16384-entry fp32 pred table (replicated per partition), producing o
    in "wrapped stream" layout (16x replicated per 16-partition group).
  - 16 accumulating matmuls de-group the stream straight into natural
    partition rows: W_q[p, i] = 1/16 iff i%16==q and p//16==i//16, so
    psum[i, j] = o of pixel (i, j).
  - Moments via 10 running sums (basis [1, x, x^2, min(x,1)] per side):
      ACT: o psum->bf16 copy (+Sum o), t^2 (+Sum), o^2 (+Sum)
      DVE: t i16->bf16 conv (+Sum t), u=(t==o) (+Sum), u*o (+Sum),
           u*o^2 (+Sum), min(t,1)/min(o,1)/min(u*o,1) sums (4x mode)
  - Host inverts the tiny 4x4 systems to get 4-bin counts, then dice.
"""

import os
import sys

sys.path.insert(0, "/opt/trn_rl_repo")
os.environ["BY_DEFAULT_DISABLE_SUBTILE_DEPS"] = "1"

from contextlib import ExitStack

import numpy as np

import concourse.bass as bass
import concourse.tile as tile
from concourse import bacc, mybir

NCORES = 8
V = 16384
NCLS = 4
N = 4096
ROWS = N // NCORES            # 512 rows per core
PIX = ROWS * N                # 2097152 pixels per core
PPART = PIX // 128            # 16384 pixels per partition
FT = 512                      # free slots per tile
NT = PPART // FT              # 32 tiles
NIDX = 16 * FT                # 8192 stream indices per gather
NMOM = 10

i32 = mybir.dt.int32
i16 = mybir.dt.int16
f32 = mybir.dt.float32
bf16 = mybir.dt.bfloat16


def _build_program():
    nc = bacc.Bacc(
        "TRN2", target_bir_lowering=False, debug=False, num_devices=NCORES
    )
    outp = nc.dram_tensor("outp", [128, 128, NCLS], f32, kind="ExternalInput")
    targ = nc.dram_tensor("targ", [128, PPART], i32, kind="ExternalInput")
    segs = nc.dram_tensor("segs", [128, PPART], i32, kind="ExternalInput")
    wde = nc.dram_tensor("wde", [128, 16 * 128], bf16, kind="ExternalInput")
    mom = nc.dram_tensor("mom", [128, NMOM], f32, kind="ExternalOutput")

    with tile.TileContext(nc) as tc:
        with ExitStack() as ctx:
            _kernel(ctx, tc, nc, outp, targ, segs, wde, mom)

    nc.compile()
    return nc


def _kernel(ctx, tc, nc, outp, targ, segs, wde, mom):
    from concourse.alu_op_type import AluOpType as Op

    Act = mybir.ActivationFunctionType

    const_pool = ctx.enter_context(tc.tile_pool(name="const", bufs=1))
    dram_pool = ctx.enter_context(tc.tile_pool(name="dram", bufs=1, space="DRAM"))
    pred_pool = ctx.enter_context(tc.tile_pool(name="predp", bufs=2))
    in_pool = ctx.enter_context(tc.tile_pool(name="inp", bufs=3))
    seg_pool = ctx.enter_context(tc.tile_pool(name="segp", bufs=3))
    stream_pool = ctx.enter_context(tc.tile_pool(name="stream", bufs=2))
    nat_pool = ctx.enter_context(tc.tile_pool(name="nat", bufs=3))
    tmp_pool = ctx.enter_context(tc.tile_pool(name="tmp", bufs=2))
    psum_pool = ctx.enter_context(tc.tile_pool(name="ps", bufs=2, space="PSUM"))

    # ---- Phase 0: pred = argmax(output, axis=1), built into a gather table --
    o_all = pred_pool.tile([128, 128, NCLS], f32)
    nc.sync.dma_start(o_all, outp.ap())

    best = pred_pool.tile([128, 128, 1], f32, tag="best")
    pred = pred_pool.tile([128, 128, 1], i32, tag="pred")
    nc.vector.tensor_copy(best, o_all[:, :, 0:1])
    nc.vector.memset(pred, 0)
    for c in range(1, NCLS):
        oc = o_all[:, :, c : c + 1]
        gt = pred_pool.tile([128, 128, 1], i32, tag="gt")
        nc.vector.tensor_tensor(gt, oc, best, Op.is_gt)
        cst = pred_pool.tile([128, 128, 1], i32, tag="cst")
        nc.vector.memset(cst, c)
        nc.vector.copy_predicated(pred, gt, cst)
        best2 = pred_pool.tile([128, 128, 1], f32, tag="best")
        nc.vector.tensor_tensor(best2, best, oc, Op.max)
        best = best2

    predf = pred_pool.tile([128, 128, 1], f32, tag="predf")
    nc.vector.tensor_copy(predf, pred)
    pred_scr = dram_pool.tile([128, 128], f32)
    nc.sync.dma_start(pred_scr, predf)

    # Broadcast the 16384-entry table into every partition (stride-0 source).
    tbl = const_pool.tile([128, V], f32)
    scr_flat = bass.AP(pred_scr.tensor, pred_scr.offset, [[0, 128], [1, V]])
    nc.sync.dma_start(tbl, scr_flat)

    # De-group weights (host-built constant): W_q[p, i] = 1/16 where
    # i % 16 == q and p // 16 == i // 16  -> psum rows are natural.
    wtile = const_pool.tile([128, 16 * 128], bf16)
    nc.sync.dma_start(wtile, wde.ap())
    wdes = [wtile[:, 128 * q : 128 * (q + 1)] for q in range(16)]

    # ---- Accumulator strip: one fp32 column per (moment, tile) -------------
    acc = const_pool.tile([128, NMOM * NT], f32)

    # ---- Phase 1: main loop ------------------------------------------------
    for it in range(NT):
        t32 = in_pool.tile([128, FT], i32, tag="t32")
        nc.sync.dma_start(t32, targ.ap()[:, it * FT : (it + 1) * FT])
        s32 = in_pool.tile([128, FT], i32, tag="s32")
        nc.sync.dma_start(s32, segs.ap()[:, it * FT : (it + 1) * FT])

        # GPSIMD narrows to int16 (values < 16384 so the low half is exact).
        s16v = s32.bitcast(i16).rearrange("p (e two) -> p e two", two=2)
        seg16 = seg_pool.tile([128, FT], i16, tag="seg16")
        nc.gpsimd.tensor_copy(seg16, s16v[:, :, 0:1])
        t16v = t32.bitcast(i16).rearrange("p (e two) -> p e two", two=2)
        t16 = seg_pool.tile([128, FT], i16, tag="t16")
        nc.gpsimd.tensor_copy(t16, t16v[:, :, 0:1])

        ostr = stream_pool.tile([128, NIDX], f32, tag="ostr")
        nc.gpsimd.ap_gather(
            ostr, tbl, seg16, channels=128, num_elems=V, d=1, num_idxs=NIDX
        )

        # De-group: 16 accumulating matmuls put o into natural psum rows.
        ostr_bf = ostr.bitcast(bf16).rearrange("p (s x) -> p s x", x=32)
        psq = psum_pool.tile([128, FT], f32, tag="psq")
        for q in range(16):
            nc.tensor.matmul(
                psq,
                wdes[q],
                ostr_bf[:, :, 2 * q + 1 : 2 * q + 2],
                start=(q == 0),
                stop=(q == 15),
            )

        def a(m):
            k = m * NT + it
            return acc[:, k : k + 1]

        # ---- bf16 conversions (fused accumulation: Sum t, Sum o) ----
        t_bf = nat_pool.tile([128, FT], bf16, tag="tbf")
        nc.vector.tensor_scalar(
            t_bf, t16, 0.0, None, Op.add, Op.add, accum_out=a(0)
        )
        o_bf = nat_pool.tile([128, FT], bf16, tag="obf")
        nc.scalar.activation(o_bf, psq, Act.Copy, accum_out=a(4))

        # ---- squares on ACT (Sum t^2, Sum o^2) ----
        wt2 = tmp_pool.tile([128, FT], bf16, tag="w", bufs=6)
        nc.scalar.activation(wt2, t_bf, Act.Square, accum_out=a(1))
        wo2 = tmp_pool.tile([128, FT], bf16, tag="w", bufs=6)
        nc.scalar.activation(wo2, o_bf, Act.Square, accum_out=a(5))

        # ---- min(x, 1) sums on DVE (4x mode) ----
        wmt = tmp_pool.tile([128, FT], bf16, tag="w", bufs=6)
        nc.vector.tensor_scalar(
            wmt, t_bf, 1.0, None, Op.min, Op.add, accum_out=a(2)
        )
        wmo = tmp_pool.tile([128, FT], bf16, tag="w", bufs=6)
        nc.vector.tensor_scalar(
            wmo, o_bf, 1.0, None, Op.min, Op.add, accum_out=a(6)
        )

        # ---- joint moments on DVE ----
        u = nat_pool.tile([128, FT], bf16, tag="u")
        nc.vector.scalar_tensor_tensor(
            u, t_bf, 0.0, o_bf, Op.bypass, Op.is_equal, accum_out=a(3)
        )
        uo = nat_pool.tile([128, FT], bf16, tag="uo")
        nc.vector.scalar_tensor_tensor(
            uo, u, 0.0, o_bf, Op.bypass, Op.mult, accum_out=a(7)
        )
        wu2 = tmp_pool.tile([128, FT], bf16, tag="w", bufs=6)
        nc.vector.scalar_tensor_tensor(
            wu2, uo, 0.0, o_bf, Op.bypass, Op.mult, accum_out=a(8)
        )
        wum = tmp_pool.tile([128, FT], bf16, tag="w", bufs=6)
        nc.vector.tensor_scalar(
            wum, uo, 1.0, None, Op.min, Op.add, accum_out=a(9)
        )

    # ---- Phase 2: fold the per-tile partials and ship out ------------------
    mom_sb = const_pool.tile([128, NMOM], f32)
    for m in range(NMOM):
        nc.vector.tensor_reduce(
            mom_sb[:, m : m + 1],
            acc[:, m * NT : (m + 1) * NT],
            mybir.AxisListType.X,
            Op.add,
        )
    nc.sync.dma_start(mom.ap(), mom_sb)


_program = None


def _get_program():
    global _program
    if _program is None:
        _program = _build_program()
    return _program


def _make_in_maps(output, target, segments):
    in_maps = []
    for c in range(NCORES):
        tblk = np.ascontiguousarray(target[c * ROWS : (c + 1) * ROWS]).reshape(
            128, PPART
        )
        sblk = np.ascontiguousarray(segments[c * ROWS : (c + 1) * ROWS]).reshape(
            128, PPART
        )
        in_maps.append(
            {
                "outp": np.ascontiguousarray(output).reshape(128, 128, NCLS),
                "targ": tblk,
                "segs": sblk,
                "wde": _wde_const(),
            }
        )
    return in_maps


_wde_cache = None


def _wde_const():
    global _wde_cache
    if _wde_cache is None:
        import ml_dtypes

        w = np.zeros((128, 16, 128), dtype=np.float32)
        for q in range(16):
            for i in range(q, 128, 16):
                g = i // 16
                w[16 * g : 16 * (g + 1), q, i] = 1.0 / 16.0
        _wde_cache = w.reshape(128, 16 * 128).astype(ml_dtypes.bfloat16)
    return _wde_cache


# Basis matrix: rows are sums of [1, c, c^2, min(c,1)] over classes c=0..3.
_M = np.array(
    [
        [1.0, 1.0, 1.0, 1.0],
        [0.0, 1.0, 2.0, 3.0],
        [0.0, 1.0, 4.0, 9.0],
        [0.0, 1.0, 1.0, 1.0],
    ]
)


def _score_from_moments(s, p_total):
    # s: (10,) float64 summed over cores and partitions
    st = np.array([p_total, s[0], s[1], s[2]])
    so = np.array([p_total, s[4], s[5], s[6]])
    su = np.array([s[3], s[7], s[8], s[9]])
    nt = np.linalg.solve(_M, st)
    no = np.linalg.solve(_M, so)
    ju = np.linalg.solve(_M, su)
    score = 2.0 * ju / (nt + no + 1e-10)
    return score.astype(np.float32)


def kernel(output, target, segments):
    from concourse.bass_utils import run_bass_kernel_spmd

    nc = _get_program()
    in_maps = _make_in_maps(output, target, segments)
    res = run_bass_kernel_spmd(nc, in_maps, core_ids=list(range(NCORES)))
    s = np.zeros(NMOM, dtype=np.float64)
    for core_out in res.results:
        s += core_out["mom"].astype(np.float64).sum(axis=0)
    return _score_from_moments(s, float(NCORES * PIX))
